# revision 2
# baseline (speedup 1.0000x reference)
"""Trainium2 Bass kernel v2: full softmax attention, 3-engine-balanced pipeline.

Math: ring attention with sigmoid/logsigmoid merge == plain softmax attention.
out[b,q,h,:] = softmax(Q K^T / sqrt(D)) V per head. B=1, S=4096, H=16, D=128.

Sharding: 2 heads per core (16 heads / 8 cores), no cross-core comms.

Per-core: 16 superblock jobs (2 heads x 8 q-superblocks of 512). Each job:
  16 k-tile pairs; per pair: QK (2 mm -> psum [128,1024]), exp (ACT for 12
  pairs, DVE Schraudolph tensor_scalar for 4), PV (2 mm, accumulated in ot).
  P-sum tree (for l = sum_k P): L0a mid-job on DVE; L0b/L1 on DVE and
  L2/L3/L4 on Pool, software-pipelined into the NEXT job's pair loop so the
  PE never waits; l = ones^T root (1 mm) lands at the end of the next job,
  l drain two jobs later.  out stays [d,q] fp32 + l[q]; host divides and
  transposes during unshard.

PSUM: 3x score-pair [128,1024] (6 banks) + ot [128,512] (1) + l [128,512] (1).
Per-engine steady budget ~14us per job, PE-bound (64 matmuls @ ~216ns).
"""

import numpy as np
import ml_dtypes
from contextlib import ExitStack

import concourse.bass as bass
import concourse.bacc as bacc
import concourse.mybir as mybir
import concourse.tile as tile
from concourse.bass_utils import run_bass_kernel_spmd

B, S, H, D = 1, 4096, 16, 128
N_CORES = 8
HPC = H // N_CORES
SB = 512                  # q superblock
NSB = S // SB             # 8 per head
NPAIR = 16                # k-tile pairs per superblock (32 k-tiles of 128)
NJOB = HPC * NSB          # 16
SCALE = float(1.0 / np.sqrt(D))
BF16 = mybir.dt.bfloat16
FP32 = mybir.dt.float32
I16 = mybir.dt.int16

# Schraudolph exp in bf16-bit space: bits16 = rint(s*C1 + C2); value =
# bitcast_bf16(bits16) ~= exp(s*SCALE), rel rms ~1.8%. C2 calibrated (rint).
LOG2E = float(np.log2(np.e))
C1 = SCALE * LOG2E * 128.0
C2 = 16248.6

# Engine of each pair-exp (True = ACT, False = DVE Schraudolph).
EXP_PATTERN = [(pi % 4) != 3 for pi in range(NPAIR)]

_CACHE = {}


def _build():
    nc = bacc.Bacc("TRN2", target_bir_lowering=False, debug=False)
    qt_d = nc.dram_tensor("qt", [HPC, 128, S], BF16, kind="ExternalInput")
    kt_d = nc.dram_tensor("kt", [HPC, 128, S], BF16, kind="ExternalInput")
    vp_d = nc.dram_tensor("vp", [HPC, 128, S], BF16, kind="ExternalInput")
    o_d = nc.dram_tensor("o", [HPC, NSB, 128, SB], FP32, kind="ExternalOutput")
    l_d = nc.dram_tensor("l", [HPC, NSB, 1, SB], FP32, kind="ExternalOutput")

    jobs = [(h, sb) for h in range(HPC) for sb in range(NSB)]

    with ExitStack() as ctx:
        tc = ctx.enter_context(tile.TileContext(nc))
        const = ctx.enter_context(tc.tile_pool(name="const", bufs=1))
        ones = const.tile([128, 1], BF16, name="ones")
        nc.gpsimd.memset(ones, 1.0)

        qkv = ctx.enter_context(tc.tile_pool(name="qkv", bufs=2))
        pbufp = ctx.enter_context(tc.tile_pool(name="pbufp", bufs=2))
        treep = ctx.enter_context(tc.tile_pool(name="treep", bufs=2))
        drainp = ctx.enter_context(tc.tile_pool(name="drainp", bufs=2))

        scp = ctx.enter_context(tc.tile_pool(name="scp", bufs=3, space="PSUM"))
        otp = ctx.enter_context(tc.tile_pool(name="otp", bufs=1, space="PSUM"))
        lp = ctx.enter_context(tc.tile_pool(name="lp", bufs=1, space="PSUM"))

        qkv_tiles = {}

        def load_head(h):
            qt_s = qkv.tile([128, S], BF16, name=f"qt{h}", tag="qt")
            kt_s = qkv.tile([128, S], BF16, name=f"kt{h}", tag="kt")
            v_s = qkv.tile([128, S], BF16, name=f"v{h}", tag="v")
            c0 = slice(0, S // 4)
            nc.sync.dma_start(kt_s[:, c0], kt_d[h][:, c0])
            nc.sync.dma_start(qt_s[:, c0], qt_d[h][:, c0])
            nc.sync.dma_start(v_s[:, c0], vp_d[h][:, c0])
            for ch in range(1, 4):
                cs = slice(ch * (S // 4), (ch + 1) * (S // 4))
                nc.sync.dma_start(kt_s[:, cs], kt_d[h][:, cs])
                nc.sync.dma_start(qt_s[:, cs], qt_d[h][:, cs])
                nc.sync.dma_start(v_s[:, cs], vp_d[h][:, cs])
            qkv_tiles[h] = (qt_s, kt_s, v_s)


        st = {}  # per-job state for the pipelined epilogue

        def epi_ot_drain(g):
            s = st[g]
            h, sb = jobs[g]
            ot_sb = drainp.tile([128, SB], FP32, name=f"od_{g}", tag="od")
            nc.vector.tensor_copy(ot_sb, s["ot"])
            nc.sync.dma_start(o_d[h, sb], ot_sb)

        def epi_l0a(g):
            s = st[g]
            nc.vector.tensor_add(
                s["t1"][:, 0:4096],
                s["pbuf"][:, 0:4096], s["pbuf"][:, 4096:8192],
            )

        def epi_l0b(g):
            s = st[g]
            nc.vector.tensor_add(
                s["t1"][:, 4096:8192],
                s["pbuf"][:, 8192:12288], s["pbuf"][:, 12288:16384],
            )

        def epi_l1(g):
            s = st[g]
            t2 = treep.tile([128, 4096], BF16, name=f"t2_{g}", tag="t2", bufs=1)
            nc.vector.tensor_add(t2, s["t1"][:, 0:4096], s["t1"][:, 4096:8192])
            s["t2"] = t2

        def epi_l2(g):
            s = st[g]
            t3 = treep.tile([128, 2048], BF16, name=f"t3_{g}", tag="t3", bufs=1)
            nc.gpsimd.tensor_add(t3, s["t2"][:, 0:2048], s["t2"][:, 2048:4096])
            s["t3"] = t3

        def epi_l3(g):
            s = st[g]
            t4 = treep.tile([128, 1024], BF16, name=f"t4_{g}", tag="t4", bufs=1)
            nc.gpsimd.tensor_add(t4, s["t3"][:, 0:1024], s["t3"][:, 1024:2048])
            s["t4"] = t4

        def epi_l4(g):
            s = st[g]
            rt = treep.tile([128, 512], BF16, name=f"rt_{g}", tag="rt", bufs=1)
            nc.gpsimd.tensor_add(rt, s["t4"][:, 0:512], s["t4"][:, 512:1024])
            s["rt"] = rt

        def epi_lmm(g):
            s = st[g]
            lt = lp.tile([128, SB], FP32, name=f"lt_{g}", tag="lt")
            nc.tensor.matmul(lt[0:1, :], ones, s["rt"], start=True, stop=True)
            s["lt"] = lt

        def lmm_direct(g, src_key, width):
            s = st[g]
            lt = lp.tile([128, SB], FP32, name=f"lt_{g}", tag="lt")
            n = width // 512
            for u in range(n):
                nc.tensor.matmul(
                    lt[0:1, :], ones, s[src_key][:, u * 512:(u + 1) * 512],
                    start=(u == 0), stop=(u == n - 1),
                )
            s["lt"] = lt

        def epi_ldrain(g):
            s = st[g]
            h, sb = jobs[g]
            l_sb = drainp.tile([1, SB], FP32, name=f"ld_{g}", tag="ld")
            nc.vector.tensor_copy(l_sb, s["lt"][0:1, :])
            nc.sync.dma_start(l_d[h, sb], l_sb)

        def emit_pv(t):
            # PV of global pair t (trails the QK stream by PV_LAG pairs)
            g, pi = divmod(t, NPAIR)
            s = st[g]
            v_s = qkv_tiles[jobs[g][0]][2]
            po = pi * 1024
            k0 = (2 * pi) * 128
            k1 = (2 * pi + 1) * 128
            nc.tensor.matmul(
                s["ot"], v_s[:, k0:k0 + 128], s["pbuf"][:, po:po + 512],
                start=(pi == 0), stop=False,
            )
            nc.tensor.matmul(
                s["ot"], v_s[:, k1:k1 + 128], s["pbuf"][:, po + 512:po + 1024],
                start=False, stop=(pi == NPAIR - 1),
            )

        PV_LAG = 4
        NT = NJOB * NPAIR
        load_head(0)
        for t in range(NT):
            g, pi = divmod(t, NPAIR)
            h, sb = jobs[g]
            if pi == 0:
                if g + 2 < NJOB and jobs[g + 2][0] != h:
                    load_head(jobs[g + 2][0])
                s = st[g] = {}
                s["ot"] = otp.tile([128, SB], FP32, name=f"ot_{g}", tag="ot")
                s["pbuf"] = pbufp.tile(
                    [128, NPAIR * 1024], BF16, name=f"pb_{g}", tag="pb"
                )
                s["pbuf_i16"] = s["pbuf"].bitcast(I16)
                s["t1"] = treep.tile([128, 8192], BF16, name=f"t1_{g}", tag="t1")
            else:
                s = st[g]
            qt_s, kt_s, v_s = qkv_tiles[h]
            q0 = sb * SB

            # epilogue injections (previous jobs)
            if pi == 4 and g >= 1:
                epi_ot_drain(g - 1)
            elif pi == 5 and g >= 1:
                epi_l0b(g - 1)
            elif pi == 8 and g >= 3:
                epi_ldrain(g - 3)
            elif pi == 9 and g >= 1:
                epi_l1(g - 1)
            elif pi == 10 and g >= 1:
                epi_l2(g - 1)
            elif pi == 13 and g != NJOB - 1:
                epi_l0a(g)
            elif pi == 14 and g >= 1:
                epi_l3(g - 1)
            elif pi == 15 and g >= 1:
                epi_l4(g - 1)

            k0 = (2 * pi) * 128
            k1 = (2 * pi + 1) * 128
            sc = scp.tile([128, 1024], FP32, name=f"sc_{g}_{pi}", tag="sc")
            nc.tensor.matmul(
                sc[:, 0:512], kt_s[:, k0:k0 + 128], qt_s[:, q0:q0 + SB],
                start=True, stop=True,
            )
            nc.tensor.matmul(
                sc[:, 512:1024], kt_s[:, k1:k1 + 128], qt_s[:, q0:q0 + SB],
                start=True, stop=True,
            )
            po = pi * 1024
            if pi % 4 == 0:
                nc.vector.tensor_scalar(
                    s["pbuf_i16"][:, po:po + 1024], sc, C1, C2,
                    op0=mybir.AluOpType.mult, op1=mybir.AluOpType.add,
                )
            else:
                nc.scalar.activation(
                    s["pbuf"][:, po:po + 1024], sc,
                    mybir.ActivationFunctionType.Exp, scale=SCALE,
                )
            if t >= PV_LAG:
                emit_pv(t - PV_LAG)
            if pi == 15 and g >= 2:
                epi_lmm(g - 2)

        # ---- tail ----
        for t in range(NT - PV_LAG, NT):
            emit_pv(t)
        gl = NJOB - 1
        epi_ot_drain(gl)
        epi_ldrain(gl - 2)
        epi_lmm(gl - 1)
        epi_ldrain(gl - 1)
        lmm_direct(gl, "pbuf", 16384)
        epi_ldrain(gl)
    nc.compile()
    return nc


def _prep_inputs(q, k, v):
    bf = ml_dtypes.bfloat16
    in_maps = []
    for c in range(N_CORES):
        hs = slice(c * HPC, (c + 1) * HPC)
        qt = np.ascontiguousarray(np.transpose(q[:, hs, :], (1, 2, 0))).astype(bf)
        kt = np.ascontiguousarray(np.transpose(k[:, hs, :], (1, 2, 0))).astype(bf)
        vh = np.transpose(v[:, hs, :], (1, 0, 2))              # [HPC, S, D]
        vp = np.ascontiguousarray(
            vh.reshape(HPC, S // 128, 128, D).transpose(0, 2, 1, 3)
        ).reshape(HPC, 128, S).astype(bf)
        in_maps.append({"qt": qt, "kt": kt, "vp": vp})
    return in_maps


def kernel(q, k, v, ring_size=None, **_unused):
    q = np.asarray(q, dtype=np.float32).reshape(S, H, D)
    k = np.asarray(k, dtype=np.float32).reshape(S, H, D)
    v = np.asarray(v, dtype=np.float32).reshape(S, H, D)

    in_maps = _prep_inputs(q, k, v)
    if "nc" not in _CACHE:
        _CACHE["nc"] = _build()
    res = run_bass_kernel_spmd(_CACHE["nc"], in_maps, list(range(N_CORES))).results

    out = np.empty((B, S, H, D), np.float32)
    for c in range(N_CORES):
        o = np.asarray(res[c]["o"])      # [HPC, NSB, 128, SB] = [h, sb, d, q]
        l = np.asarray(res[c]["l"]).reshape(HPC, NSB, SB)
        for hh in range(HPC):
            od = np.transpose(o[hh], (0, 2, 1)).reshape(S, D)  # [q, d]
            out[0, :, c * HPC + hh, :] = od / l[hh].reshape(S, 1)
    return out
